# revision 39
# baseline (speedup 1.0000x reference)
"""Trainium2 Bass kernel for nn_Conv2d_71476845740806.

Reference semantics (buggy naive Conv2d):
  xsum = pad(input, 1).sum(batch)                  # (1, C, 258, 258)
  conv = conv2d(xsum, weight, stride=2, VALID)     # (1, K, 128, 128)
  vals = conv[0, :, :64, :64]                      # (K, 64, 64)
  out  = zeros(B, K, 128, 128); out[:, :, ::2, ::2] = vals  (batch-replicated)

Only window starts (2i, 2j), i,j in [0,64) are used -> only padded rows/cols
0..128 of the summed image matter -> only input rows/cols 0..127.

Device strategy (8 cores, SPMD):
  - Shard the 64 output rows: core q computes rows 8q..8q+7 for ALL K=128
    filters. Needs padded rows 16q..16q+16 (17 rows) x 129 cols, all b, c.
  - Host preps 8 per-core per-batch fp16 tensors x_b[128, 1170]:
    partitions 0..63 = (c, even padded rows 0..8 x 130 cols),
    partitions 64..127 = (c, odd padded rows 0..7 x 130 cols, zero-padded).
    Each is its own contiguous DRAM tensor -> sequential DRAM reads.
  - fp16 halves the HBM traffic vs f32 (rel err ~6e-4, threshold 2e-2).
  - Tree batch-sum: 7 chained DVE adds (ping-pong fp16 accumulators), each
    hidden between chunk DMA arrivals. One semaphore per chunk DMA (a DMA's
    +16 completion is spread over 16 engines and consecutive DMAs overlap
    across engines, so a shared counter can fire on mixed partials).
  - Conv as ONE accumulation group of 6 matmuls into PSUM [128, 512]:
      3 x contract-128 (kernel rows dh=0,1 paired across partition halves)
      3 x contract-64  (dh=2, even partitions only, shifted one row)
    rhs AP does the stride-2 column access directly.
  - PE p-state: the tensor engine only reaches full clock after ~3us of
    gap-free execution. A run of back-to-back throwaway matmuls (each into
    its own PSUM slice, on landed chunk-0 data) bridges the add chain so
    the 6 real matmuls issue with zero idle gap at full clock.
  - DVE casts PSUM -> SBUF fp16; DMA out 128x512 fp16; host casts to f32
    and scatters into the zero output.
"""

import contextlib

import ml_dtypes
import numpy as np

import concourse.bacc as bacc
import concourse.bass as bass
import concourse.mybir as mybir
from concourse import tile
from concourse.bass_utils import run_bass_kernel_spmd

F32 = mybir.dt.float32
FP16 = mybir.dt.float16

B, C, H, W = 8, 64, 256, 256
K = 128
NCORES = 8
ROWS_PER_CORE = 8          # output rows per core (64 total)
ER = 9                     # even padded rows per core
OR = 8                     # odd padded rows per core
WCOLS = 130                # stored padded cols 0..129 (used: 0..128)
PITCH = ER * WCOLS         # 1170 per-batch free pitch
NOUT = ROWS_PER_CORE * 64  # 512

# knobs
WARMUP = 50                # back-to-back warmup matmuls (0=off)
WFREE = 512                # free size of each warmup matmul
NQUEUES = 1                # input chunk DMA queues (1=sync only, 2=+scalar)
SPLIT_LAST = True          # split chunk 7's DMA + final add into halves
NOWAIT_OUT = True          # don't hold block end on the out DMA completion
PRIME_DMA = True           # tiny leading DMA to absorb sync-queue init cost

TRACE = False
LAST_EXEC_NS = None

_cache = {}


def _build_v3():
    key = (
        "v5", WARMUP, WFREE, NQUEUES, SPLIT_LAST,
        NOWAIT_OUT, PRIME_DMA,
    )
    if key in _cache:
        return _cache[key]

    nc = bacc.Bacc(None)
    xcs = [
        nc.declare_dram_parameter(f"x{b}", [128, PITCH], FP16, isOutput=False)
        for b in range(B)
    ]
    wc1 = nc.declare_dram_parameter("wc1", [128, 3 * K], FP16, isOutput=False)
    wc2 = nc.declare_dram_parameter("wc2", [64, 3 * K], FP16, isOutput=False)
    out = nc.declare_dram_parameter("out", [K, NOUT], FP16, isOutput=True)

    ctx = contextlib.ExitStack()
    wt = ctx.enter_context(nc.sbuf_tensor([128, 3 * K], FP16))
    wt2 = ctx.enter_context(nc.sbuf_tensor([64, 3 * K], FP16))
    staging = ctx.enter_context(nc.sbuf_tensor([128, B * PITCH], FP16))
    acc0 = ctx.enter_context(nc.sbuf_tensor([128, PITCH], FP16))
    acc1 = ctx.enter_context(nc.sbuf_tensor([128, PITCH], FP16))
    outs = ctx.enter_context(nc.sbuf_tensor([K, NOUT], FP16))
    # Zeroed once by gpsimd at block entry; warmup matmuls read it so their
    # only dependency is the immediately-satisfied memset semaphore.
    junk = ctx.enter_context(nc.sbuf_tensor([128, 128 + WFREE], FP16))
    scratch = ctx.enter_context(nc.sbuf_tensor([1, 16], FP16))
    psum = ctx.enter_context(nc.psum_tensor("psum", [K, NOUT], F32))
    NSLOT = 3 * (1024 // WFREE)  # warmup PSUM slots (recycled through dwu_sem)
    dpsums = [
        ctx.enter_context(nc.psum_tensor(f"dpsum{i}", [K, 1024], F32))
        for i in range(3 if WARMUP else 0)
    ]
    nsem = B + 1 if SPLIT_LAST else B
    in_sems = [ctx.enter_context(nc.semaphore(f"in_sem{b}")) for b in range(nsem)]
    w_sem = ctx.enter_context(nc.semaphore("w_sem"))
    w2_sem = ctx.enter_context(nc.semaphore("w2_sem"))
    add_sem = ctx.enter_context(nc.semaphore("add_sem"))
    mm_sem = ctx.enter_context(nc.semaphore("mm_sem"))
    cp_sem = ctx.enter_context(nc.semaphore("cp_sem"))
    odma_sem = ctx.enter_context(nc.semaphore("odma_sem"))
    dwu_sem = ctx.enter_context(nc.semaphore("dwu_sem")) if WARMUP else None
    js_sem = ctx.enter_context(nc.semaphore("js_sem")) if WARMUP else None
    prime_sem = ctx.enter_context(nc.semaphore("prime_sem")) if PRIME_DMA else None

    wpt3 = wt[:, :].rearrange("p (a b) -> p a b", a=3)
    w2t3 = wt2[:, :].rearrange("p (a b) -> p a b", a=3)
    st3 = staging[:, :].rearrange("p (b x) -> p b x", b=B)
    psum3 = psum[:, :].rearrange("p (r w) -> p r w", r=ROWS_PER_CORE)
    accs = [acc0, acc1]

    with nc.Block() as block:

        @block.gpsimd
        def _(g):
            # Weights first: their transfer rides the input queue's doorbell
            # dead-window instead of stealing engine time mid-stream.
            g.dma_start(out=wt[:, :], in_=wc1[:, :]).then_inc(w_sem, 16)
            g.dma_start(out=wt2[:, :], in_=wc2[:, :]).then_inc(w2_sem, 16)
            if WARMUP:
                g.memset(junk[:, :], 0).then_inc(js_sem, 1)

        HALF = PITCH // 2  # 585

        @block.sync
        def _(sync):
            if PRIME_DMA:
                # 1-partition no-op transfer: pays the queue-init cost before
                # chunk 0's descriptors instead of inside them.
                sync.dma_start(out=scratch[0:1, 0:1], in_=xcs[0][0:1, 0:1]).then_inc(
                    prime_sem, 16
                )
            for ch in range(B):
                if NQUEUES == 2 and ch % 2 == 1:
                    continue
                if SPLIT_LAST and ch == B - 1:
                    sync.dma_start(
                        out=st3[:, ch : ch + 1, 0:HALF], in_=xcs[ch][:, 0:HALF]
                    ).then_inc(in_sems[ch], 16)
                    sync.dma_start(
                        out=st3[:, ch : ch + 1, HALF:PITCH],
                        in_=xcs[ch][:, HALF:PITCH],
                    ).then_inc(in_sems[ch + 1], 16)
                else:
                    sync.dma_start(
                        out=st3[:, ch : ch + 1, :], in_=xcs[ch][:, :]
                    ).then_inc(in_sems[ch], 16)
            sync.wait_ge(cp_sem, 1)
            sync.dma_start(out=out[:, :], in_=outs[:, :]).then_inc(odma_sem, 16)
            if not NOWAIT_OUT:
                # With NOWAIT_OUT the fixed multi-microsecond teardown behind
                # block end covers the out transfer instead.
                sync.wait_ge(odma_sem, 16)

        if NQUEUES == 2:

            @block.scalar
            def _(s):
                for ch in range(1, B, 2):
                    s.dma_start(
                        out=st3[:, ch : ch + 1, :], in_=xcs[ch][:, :]
                    ).then_inc(in_sems[ch], 16)

        @block.vector
        def _(v):
            v.wait_ge(in_sems[0], 16)
            v.wait_ge(in_sems[1], 16)
            v.tensor_add(acc0[:, :], st3[:, 0, :], st3[:, 1, :]).then_inc(add_sem, 1)
            nadd = 1
            for i in range(2, B):
                dst = accs[(i + 1) % 2]
                src = accs[i % 2]
                if SPLIT_LAST and i == B - 1:
                    for h in range(2):
                        c0, c1 = (0, HALF) if h == 0 else (HALF, PITCH)
                        v.wait_ge(in_sems[i + h], 16)
                        v.wait_ge(add_sem, nadd)
                        v.tensor_add(
                            dst[:, c0:c1], src[:, c0:c1], st3[:, i, c0:c1]
                        ).then_inc(add_sem, 1)
                        nadd += 1
                else:
                    v.wait_ge(in_sems[i], 16)
                    v.wait_ge(add_sem, nadd)
                    v.tensor_add(
                        dst[:, :], src[:, :], st3[:, i, :]
                    ).then_inc(add_sem, 1)
                    nadd += 1
            v.wait_ge(mm_sem, 1)
            v.tensor_copy(outs[:, :], psum[:, :]).then_inc(cp_sem, 1)

        NADDS = B if SPLIT_LAST else B - 1

        @block.tensor
        def _(t):
            # Warmup run: reads never-written SBUF (no data waits), each
            # matmul targets one of NSLOT recycled PSUM slots; slot reuse is
            # ordered through dwu_sem waits on long-passed counts. The run
            # starts at block entry and executes gap-free until the real
            # matmuls, ramping the PE clock to full.
            wu_lhs = junk[:, 0:128]
            wu_rhs = junk[:, 128 : 128 + WFREE]
            for i in range(WARMUP):
                t.wait_ge(js_sem, 1)
                if i >= NSLOT:
                    t.wait_ge(dwu_sem, i - NSLOT + 1)
                s = i % NSLOT
                per = 1024 // WFREE
                dslot = dpsums[s // per][:, (s % per) * WFREE : (s % per + 1) * WFREE]
                nc.tensor.matmul(
                    dslot, wu_lhs, wu_rhs, start=True, stop=True
                ).then_inc(dwu_sem, 1)
            t.wait_ge(w_sem, 16)
            t.wait_ge(w2_sem, 16)
            t.wait_ge(add_sem, NADDS)
            src3 = accs[0][:, :].rearrange("p (r w) -> p r w", r=ER)
            for dw in range(3):
                nc.tensor.matmul(
                    psum3[:, :, :],
                    wpt3[:, dw, :],
                    src3[:, 0:ROWS_PER_CORE, dw : dw + 128 : 2],
                    start=(dw == 0),
                    stop=False,
                )
            for dw in range(3):
                mm = nc.tensor.matmul(
                    psum3[0:K, :, :],
                    w2t3[0:64, dw, :],
                    src3[0:64, 1 : 1 + ROWS_PER_CORE, dw : dw + 128 : 2],
                    start=False,
                    stop=(dw == 2),
                )
                if dw == 2:
                    mm.then_inc(mm_sem, 1)

    nc.compile()
    ctx.close()
    _cache[key] = nc
    return nc


def _prep_inputs_v3(input, weight):
    inp = np.ascontiguousarray(input, dtype=np.float32)
    w = np.ascontiguousarray(weight, dtype=np.float32)

    # Padded top-left region: P[r, w] = padded coord (orig r-1, w-1)
    P = np.zeros((B, C, 130, WCOLS), np.float16)
    P[:, :, 1:129, 1:129] = inp[:, :, :128, :128]
    Pc = np.ascontiguousarray(P.transpose(1, 0, 2, 3))  # (C, B, 130, WCOLS)

    t = [w[:, :, dh, :].transpose(1, 2, 0).reshape(-1, 3 * K) for dh in range(3)]
    wc1_host = np.empty((128, 3 * K), np.float32)
    wc1_host[0:64] = t[0]
    wc1_host[64:128] = t[1]
    wc1_host = np.ascontiguousarray(wc1_host.astype(np.float16))
    wc2_host = np.ascontiguousarray(t[2].astype(np.float16))

    in_maps = []
    for q in range(NCORES):
        r0 = 16 * q
        m = {"wc1": wc1_host, "wc2": wc2_host}
        for b in range(B):
            xb = np.zeros((128, PITCH), np.float16)
            xb[0:64] = Pc[:, b, r0 : r0 + 17 : 2, :].reshape(64, PITCH)
            xb[64:128, 0 : OR * WCOLS] = Pc[:, b, r0 + 1 : r0 + 16 : 2, :].reshape(
                64, OR * WCOLS
            )
            m[f"x{b}"] = xb
        in_maps.append(m)
    return in_maps


def kernel(input, weight):
    global LAST_EXEC_NS
    nc = _build_v3()
    in_maps = _prep_inputs_v3(input, weight)
    res = run_bass_kernel_spmd(nc, in_maps, list(range(NCORES)), trace=TRACE)
    LAST_EXEC_NS = res.exec_time_ns

    vals = np.concatenate(
        [
            res.results[q]["out"]
            .astype(np.float32)
            .reshape(K, ROWS_PER_CORE, 64)
            for q in range(NCORES)
        ],
        axis=1,
    )  # (K, 64, 64)
    out = np.zeros((B, K, 128, 128), np.float32)
    out[:, :, ::2, ::2] = vals[None]
    return out


# revision 40
# speedup vs baseline: 1.1658x; 1.1658x over previous
"""Trainium2 Bass kernel for nn_Conv2d_71476845740806.

Reference semantics (buggy naive Conv2d):
  xsum = pad(input, 1).sum(batch)                  # (1, C, 258, 258)
  conv = conv2d(xsum, weight, stride=2, VALID)     # (1, K, 128, 128)
  vals = conv[0, :, :64, :64]                      # (K, 64, 64)
  out  = zeros(B, K, 128, 128); out[:, :, ::2, ::2] = vals  (batch-replicated)

Only window starts (2i, 2j), i,j in [0,64) are used -> only padded rows/cols
0..128 of the summed image matter -> only input rows/cols 0..127.

Device strategy (8 cores, SPMD):
  - Shard the 64 output rows: core q computes rows 8q..8q+7 for ALL K=128
    filters. Needs padded rows 16q..16q+16 (17 rows) x 129 cols, all b, c.
  - Host preps 8 per-core per-batch fp16 tensors x_b[128, 1170]:
    partitions 0..63 = (c, even padded rows 0..8 x 130 cols),
    partitions 64..127 = (c, odd padded rows 0..7 x 130 cols, zero-padded).
    Each is its own contiguous DRAM tensor -> sequential DRAM reads.
  - fp16 halves the HBM traffic vs f32 (rel err ~6e-4, threshold 2e-2).
  - Tree batch-sum: 7 chained DVE adds (ping-pong fp16 accumulators), each
    hidden between chunk DMA arrivals. One semaphore per chunk DMA (a DMA's
    +16 completion is spread over 16 engines and consecutive DMAs overlap
    across engines, so a shared counter can fire on mixed partials).
  - Conv as ONE accumulation group of 6 matmuls into PSUM [128, 512]:
      3 x contract-128 (kernel rows dh=0,1 paired across partition halves)
      3 x contract-64  (dh=2, even partitions only, shifted one row)
    rhs AP does the stride-2 column access directly.
  - PE p-state: the tensor engine only reaches full clock after ~3us of
    gap-free execution. A run of back-to-back throwaway matmuls (each into
    its own PSUM slice, on landed chunk-0 data) bridges the add chain so
    the 6 real matmuls issue with zero idle gap at full clock.
  - DVE casts PSUM -> SBUF fp16; DMA out 128x512 fp16; host casts to f32
    and scatters into the zero output.
"""

import contextlib

import ml_dtypes
import numpy as np

import concourse.bacc as bacc
import concourse.bass as bass
import concourse.mybir as mybir
from concourse import tile
from concourse.bass_utils import run_bass_kernel_spmd

F32 = mybir.dt.float32
FP16 = mybir.dt.float16

B, C, H, W = 8, 64, 256, 256
K = 128
NCORES = 8
ROWS_PER_CORE = 8          # output rows per core (64 total)
ER = 9                     # even padded rows per core
OR = 8                     # odd padded rows per core
WCOLS = 130                # stored padded cols 0..129 (used: 0..128)
PITCH = ER * WCOLS         # 1170 per-batch free pitch
NOUT = ROWS_PER_CORE * 64  # 512

# knobs
WARMUP = 50                # back-to-back warmup matmuls (0=off)
WFREE = 512                # free size of each warmup matmul
NQUEUES = 1                # input chunk DMA queues (1=sync only, 2=+scalar)
SPLIT_LAST = True          # split chunk 7's DMA + final add into halves
NOWAIT_OUT = True          # don't hold block end on the out DMA completion
PRIME_DMA = True           # tiny leading DMA to absorb sync-queue init cost

TRACE = False
LAST_EXEC_NS = None

_cache = {}


def _build_v3():
    key = (
        "v5", WARMUP, WFREE, NQUEUES, SPLIT_LAST,
        NOWAIT_OUT, PRIME_DMA,
    )
    if key in _cache:
        return _cache[key]

    nc = bacc.Bacc(None)
    xcs = [
        nc.declare_dram_parameter(f"x{b}", [128, PITCH], FP16, isOutput=False)
        for b in range(B)
    ]
    wc1 = nc.declare_dram_parameter("wc1", [128, 3 * K], FP16, isOutput=False)
    wc2 = nc.declare_dram_parameter("wc2", [64, 3 * K], FP16, isOutput=False)
    out = nc.declare_dram_parameter("out", [K, NOUT], FP16, isOutput=True)

    ctx = contextlib.ExitStack()
    wt = ctx.enter_context(nc.sbuf_tensor([128, 3 * K], FP16))
    wt2 = ctx.enter_context(nc.sbuf_tensor([64, 3 * K], FP16))
    staging = ctx.enter_context(nc.sbuf_tensor([128, B * PITCH], FP16))
    acc0 = ctx.enter_context(nc.sbuf_tensor([128, PITCH], FP16))
    acc1 = ctx.enter_context(nc.sbuf_tensor([128, PITCH], FP16))
    outs = ctx.enter_context(nc.sbuf_tensor([K, NOUT], FP16))
    # Zeroed once by gpsimd at block entry; warmup matmuls read it so their
    # only dependency is the immediately-satisfied memset semaphore.
    junk = ctx.enter_context(nc.sbuf_tensor([128, 128 + WFREE], FP16))
    scratch = ctx.enter_context(nc.sbuf_tensor([1, 16], FP16))
    psum = ctx.enter_context(nc.psum_tensor("psum", [K, NOUT], F32))
    NSLOT = 3 * (1024 // WFREE)  # warmup PSUM slots (recycled through dwu_sem)
    dpsums = [
        ctx.enter_context(nc.psum_tensor(f"dpsum{i}", [K, 1024], F32))
        for i in range(3 if WARMUP else 0)
    ]
    nsem = B + 1 if SPLIT_LAST else B
    in_sems = [ctx.enter_context(nc.semaphore(f"in_sem{b}")) for b in range(nsem)]
    w_sem = ctx.enter_context(nc.semaphore("w_sem"))
    w2_sem = ctx.enter_context(nc.semaphore("w2_sem"))
    add_sem = ctx.enter_context(nc.semaphore("add_sem"))
    mm_sem = ctx.enter_context(nc.semaphore("mm_sem"))
    cp_sem = ctx.enter_context(nc.semaphore("cp_sem"))
    odma_sem = ctx.enter_context(nc.semaphore("odma_sem"))
    dwu_sem = ctx.enter_context(nc.semaphore("dwu_sem")) if WARMUP else None
    js_sem = ctx.enter_context(nc.semaphore("js_sem")) if WARMUP else None
    prime_sem = ctx.enter_context(nc.semaphore("prime_sem")) if PRIME_DMA else None

    wpt3 = wt[:, :].rearrange("p (a b) -> p a b", a=3)
    w2t3 = wt2[:, :].rearrange("p (a b) -> p a b", a=3)
    st3 = staging[:, :].rearrange("p (b x) -> p b x", b=B)
    psum3 = psum[:, :].rearrange("p (r w) -> p r w", r=ROWS_PER_CORE)
    accs = [acc0, acc1]

    with nc.Block() as block:

        @block.gpsimd
        def _(g):
            if WARMUP:
                g.memset(junk[:, :], 0).then_inc(js_sem, 1)
            g.dma_start(out=wt[:, :], in_=wc1[:, :]).then_inc(w_sem, 16)
            g.dma_start(out=wt2[:, :], in_=wc2[:, :]).then_inc(w2_sem, 16)

        HALF = PITCH // 2  # 585

        @block.sync
        def _(sync):
            if PRIME_DMA:
                # 1-partition no-op transfer: pays the queue-init cost before
                # chunk 0's descriptors instead of inside them.
                sync.dma_start(out=scratch[0:1, 0:1], in_=xcs[0][0:1, 0:1]).then_inc(
                    prime_sem, 16
                )
            for ch in range(B):
                if NQUEUES == 2 and ch % 2 == 1:
                    continue
                if SPLIT_LAST and ch == B - 1:
                    sync.dma_start(
                        out=st3[:, ch : ch + 1, 0:HALF], in_=xcs[ch][:, 0:HALF]
                    ).then_inc(in_sems[ch], 16)
                    sync.dma_start(
                        out=st3[:, ch : ch + 1, HALF:PITCH],
                        in_=xcs[ch][:, HALF:PITCH],
                    ).then_inc(in_sems[ch + 1], 16)
                else:
                    sync.dma_start(
                        out=st3[:, ch : ch + 1, :], in_=xcs[ch][:, :]
                    ).then_inc(in_sems[ch], 16)
            sync.wait_ge(cp_sem, 1)
            sync.dma_start(out=out[:, :], in_=outs[:, :]).then_inc(odma_sem, 16)
            if not NOWAIT_OUT:
                # With NOWAIT_OUT the fixed multi-microsecond teardown behind
                # block end covers the out transfer instead.
                sync.wait_ge(odma_sem, 16)

        if NQUEUES == 2:

            @block.scalar
            def _(s):
                for ch in range(1, B, 2):
                    s.dma_start(
                        out=st3[:, ch : ch + 1, :], in_=xcs[ch][:, :]
                    ).then_inc(in_sems[ch], 16)

        @block.vector
        def _(v):
            v.wait_ge(in_sems[0], 16)
            v.wait_ge(in_sems[1], 16)
            v.tensor_add(acc0[:, :], st3[:, 0, :], st3[:, 1, :]).then_inc(add_sem, 1)
            nadd = 1
            for i in range(2, B):
                dst = accs[(i + 1) % 2]
                src = accs[i % 2]
                if SPLIT_LAST and i == B - 1:
                    for h in range(2):
                        c0, c1 = (0, HALF) if h == 0 else (HALF, PITCH)
                        v.wait_ge(in_sems[i + h], 16)
                        v.wait_ge(add_sem, nadd)
                        v.tensor_add(
                            dst[:, c0:c1], src[:, c0:c1], st3[:, i, c0:c1]
                        ).then_inc(add_sem, 1)
                        nadd += 1
                else:
                    v.wait_ge(in_sems[i], 16)
                    v.wait_ge(add_sem, nadd)
                    v.tensor_add(
                        dst[:, :], src[:, :], st3[:, i, :]
                    ).then_inc(add_sem, 1)
                    nadd += 1
            v.wait_ge(mm_sem, 1)
            v.tensor_copy(outs[:, :], psum[:, :]).then_inc(cp_sem, 1)

        NADDS = B if SPLIT_LAST else B - 1

        @block.tensor
        def _(t):
            # Warmup run: reads never-written SBUF (no data waits), each
            # matmul targets one of NSLOT recycled PSUM slots; slot reuse is
            # ordered through dwu_sem waits on long-passed counts. The run
            # starts at block entry and executes gap-free until the real
            # matmuls, ramping the PE clock to full.
            wu_lhs = junk[:, 0:128]
            wu_rhs = junk[:, 128 : 128 + WFREE]
            for i in range(WARMUP):
                t.wait_ge(js_sem, 1)
                if i >= NSLOT:
                    t.wait_ge(dwu_sem, i - NSLOT + 1)
                s = i % NSLOT
                per = 1024 // WFREE
                dslot = dpsums[s // per][:, (s % per) * WFREE : (s % per + 1) * WFREE]
                nc.tensor.matmul(
                    dslot, wu_lhs, wu_rhs, start=True, stop=True
                ).then_inc(dwu_sem, 1)
            t.wait_ge(w_sem, 16)
            t.wait_ge(w2_sem, 16)
            t.wait_ge(add_sem, NADDS)
            src3 = accs[0][:, :].rearrange("p (r w) -> p r w", r=ER)
            for dw in range(3):
                nc.tensor.matmul(
                    psum3[:, :, :],
                    wpt3[:, dw, :],
                    src3[:, 0:ROWS_PER_CORE, dw : dw + 128 : 2],
                    start=(dw == 0),
                    stop=False,
                )
            for dw in range(3):
                mm = nc.tensor.matmul(
                    psum3[0:K, :, :],
                    w2t3[0:64, dw, :],
                    src3[0:64, 1 : 1 + ROWS_PER_CORE, dw : dw + 128 : 2],
                    start=False,
                    stop=(dw == 2),
                )
                if dw == 2:
                    mm.then_inc(mm_sem, 1)

    nc.compile()
    ctx.close()
    _cache[key] = nc
    return nc


def _prep_inputs_v3(input, weight):
    inp = np.ascontiguousarray(input, dtype=np.float32)
    w = np.ascontiguousarray(weight, dtype=np.float32)

    # Padded top-left region: P[r, w] = padded coord (orig r-1, w-1)
    P = np.zeros((B, C, 130, WCOLS), np.float16)
    P[:, :, 1:129, 1:129] = inp[:, :, :128, :128]
    Pc = np.ascontiguousarray(P.transpose(1, 0, 2, 3))  # (C, B, 130, WCOLS)

    t = [w[:, :, dh, :].transpose(1, 2, 0).reshape(-1, 3 * K) for dh in range(3)]
    wc1_host = np.empty((128, 3 * K), np.float32)
    wc1_host[0:64] = t[0]
    wc1_host[64:128] = t[1]
    wc1_host = np.ascontiguousarray(wc1_host.astype(np.float16))
    wc2_host = np.ascontiguousarray(t[2].astype(np.float16))

    in_maps = []
    for q in range(NCORES):
        r0 = 16 * q
        m = {"wc1": wc1_host, "wc2": wc2_host}
        for b in range(B):
            xb = np.zeros((128, PITCH), np.float16)
            xb[0:64] = Pc[:, b, r0 : r0 + 17 : 2, :].reshape(64, PITCH)
            xb[64:128, 0 : OR * WCOLS] = Pc[:, b, r0 + 1 : r0 + 16 : 2, :].reshape(
                64, OR * WCOLS
            )
            m[f"x{b}"] = xb
        in_maps.append(m)
    return in_maps


def kernel(input, weight):
    global LAST_EXEC_NS
    nc = _build_v3()
    in_maps = _prep_inputs_v3(input, weight)
    res = run_bass_kernel_spmd(nc, in_maps, list(range(NCORES)), trace=TRACE)
    LAST_EXEC_NS = res.exec_time_ns

    vals = np.concatenate(
        [
            res.results[q]["out"]
            .astype(np.float32)
            .reshape(K, ROWS_PER_CORE, 64)
            for q in range(NCORES)
        ],
        axis=1,
    )  # (K, 64, 64)
    out = np.zeros((B, K, 128, 128), np.float32)
    out[:, :, ::2, ::2] = vals[None]
    return out
